# revision 16
# baseline (speedup 1.0000x reference)
"""Depthwise 41x41 Gaussian blur (ReflectionPad2d(20)) on 8 Trainium2 cores.

Strategy:
  * The 41x41 Gaussian kernel is rank-1 (separable): w2d = outer(g, g) with
    sum(g) == 1.  We extract g = w2d.sum(axis=1) on the host and run two 1-D
    41-tap convolution passes (along H, then along W).
  * Each 1-D pass is expressed as a banded matmul: out = A^T-contract over
    input rows, where A[r, h] = sum_k g[k] [reflect(h+k-20) == r] folds the
    reflection padding into the band matrix.  A is built on the host in
    float64 from the runtime weight.
  * On-device, pass 1 uses the image as the matmul stationary operand
    (lhsT = x[r_chunk, w_tile], rhs = A[r_chunk, h_window]) producing the
    H-blurred image TRANSPOSED ([w, h] layout); pass 2 repeats the same
    structure on the transposed intermediate, which lands the final image
    back in natural [h, w] layout.  No explicit transposes anywhere.
  * Band structure: for each 128-row input chunk only a ~208-wide window of
    output columns is touched, so each PSUM accumulation group is 4 matmuls
    with clipped, overlapping N-windows (start=True clears the whole bank,
    so overwrite-then-accumulate per element is exact).
  * Sharding: data-parallel over batch*channel; 96 images / 8 cores = 12
    images per core.  The band matrix A is replicated.
"""

import numpy as np

B, C, H, W = 16, 6, 512, 512
PAD, KS = 20, 41
NCORES = 8
IMGS = (B * C) // NCORES  # 12 images per core
P = 128
NCH = H // P  # 4 chunks of 128 rows

# Matmul input dtype: "f32" (exact, 4 cyc/row) or "f32r" (TF32-like ~1.5e-4
# relative error, 1 cyc/row at N>=256).
MM_DTYPE = "f32r"

_cache = {}


def _reflect(r):
    # PyTorch ReflectionPad2d index map for pad=20 on a length-512 axis.
    if r < 0:
        return -r
    if r > H - 1:
        return 2 * (H - 1) - r
    return r


def _build_A(g):
    """A[r, h] = sum of g[k] over taps k whose padded input row reflects to r."""
    A = np.zeros((H, H), dtype=np.float64)
    for h in range(H):
        for k in range(KS):
            A[_reflect(h + k - PAD), h] += g[k]
    return A


def _windows(A, min_n):
    """Per-128-row-chunk nonzero output-column windows, padded to >= min_n."""
    wins = []
    for c in range(NCH):
        nz = np.nonzero(A[c * P:(c + 1) * P, :].any(axis=0))[0]
        lo, hi = int(nz[0]), int(nz[-1]) + 1
        if hi - lo < min_n:
            lo = max(0, hi - min_n)
            hi = min(H, lo + min_n)
            lo = max(0, hi - min_n)
        wins.append((lo, hi))
    # the union of windows must cover every output column exactly
    cover = np.zeros(H, dtype=bool)
    prev_hi = 0
    for lo, hi in wins:
        assert lo <= prev_hi, f"window gap: {wins}"
        cover[lo:hi] = True
        prev_hi = max(prev_hi, hi)
    assert cover.all(), f"windows don't cover output: {wins}"
    return wins


def _build_program(wins):
    from contextlib import ExitStack
    import concourse.tile as tile
    from concourse import bacc, mybir

    dt_in = mybir.dt.float32r if MM_DTYPE == "f32r" else mybir.dt.float32
    f32 = mybir.dt.float32

    nc = bacc.Bacc("TRN2", target_bir_lowering=False, debug=False,
                   num_devices=NCORES)
    wmax = max(hi - lo for lo, hi in wins)
    x_d = nc.dram_tensor("x", [IMGS, H, W], dt_in, kind="ExternalInput").ap()
    a_d = nc.dram_tensor("A", [NCH, P, wmax], dt_in, kind="ExternalInput").ap()
    y_d = nc.dram_tensor("y", [IMGS, H, W], f32, kind="ExternalOutput").ap()

    xr = x_d.rearrange("i (c p) w -> i p c w", p=P)
    yr = y_d.rearrange("i (c p) w -> i p c w", p=P)
    ar = a_d.rearrange("c p j -> p c j")

    with tile.TileContext(nc) as tc, ExitStack() as ctx:
        apool = ctx.enter_context(tc.tile_pool(name="a", bufs=1))
        xpool = ctx.enter_context(tc.tile_pool(name="x", bufs=6))
        tpool = ctx.enter_context(tc.tile_pool(name="t", bufs=3))
        opool = ctx.enter_context(tc.tile_pool(name="o", bufs=6))
        ps1 = ctx.enter_context(tc.tile_pool(name="ps1", bufs=4, space="PSUM"))
        ps2 = ctx.enter_context(tc.tile_pool(name="ps2", bufs=4, space="PSUM"))

        At = apool.tile([P, NCH, wmax], dt_in, tag="A")

        xt = [None] * IMGS
        tt = [None] * IMGS
        copy_idx = [0]

        def psum_copy(dst, src):
            k = copy_idx[0]
            copy_idx[0] += 1
            if k % 2 == 0:
                nc.vector.tensor_copy(dst, src)
            else:
                nc.scalar.copy(dst, src)

        def pass1(i):
            xt[i] = xpool.tile([P, NCH, W], dt_in, tag="x", name=f"x{i}")
            if i == 0:
                # chunk-interleaved first loads so the PE can start after
                # ~0.5 MB of DMA instead of 2 MB
                for c in range(NCH):
                    nc.sync.dma_start(At[:, c, :], ar[:, c, :])
                    nc.sync.dma_start(xt[i][:, c, :], xr[i][:, c, :])
            elif i == IMGS - 1:
                # chunked last load shortens the serial tail
                for c in range(NCH):
                    nc.sync.dma_start(xt[i][:, c, :], xr[i][:, c, :])
            else:
                nc.sync.dma_start(xt[i][:], xr[i])
            tt[i] = tpool.tile([P, NCH, H], dt_in, tag="t", name=f"t{i}")
            if True:
                # c-outer emission: all 4 accumulation groups open at once, so
                # only the final c's matmuls wait on the last-arriving chunk
                pts = [ps1.tile([P, H], f32, tag="p1", name=f"p1_{i}_{m}")
                       for m in range(NCH)]
                for c in range(NCH):
                    lo, hi = wins[c]
                    for m in range(NCH):
                        nc.tensor.matmul(
                            pts[m][:, lo:hi],
                            xt[i][:, c, m * P:(m + 1) * P],
                            At[:, c, 0:hi - lo],
                            start=(c == 0), stop=(c == NCH - 1),
                        )
                for m in range(NCH):
                    psum_copy(tt[i][:, m, :], pts[m][:])
            else:
                for m in range(NCH):
                    pt = ps1.tile([P, H], f32, tag="p1")
                    for c in range(NCH):
                        lo, hi = wins[c]
                        nc.tensor.matmul(
                            pt[:, lo:hi],
                            xt[i][:, c, m * P:(m + 1) * P],
                            At[:, c, 0:hi - lo],
                            start=(c == 0), stop=(c == NCH - 1),
                        )
                    psum_copy(tt[i][:, m, :], pt[:])

        def pass2(i):
            ot = opool.tile([P, NCH, W], f32, tag="o")
            for t in range(NCH):
                pt = ps2.tile([P, W], f32, tag="p2")
                for c in range(NCH):
                    lo, hi = wins[c]
                    nc.tensor.matmul(
                        pt[:, lo:hi],
                        tt[i][:, c, t * P:(t + 1) * P],
                        At[:, c, 0:hi - lo],
                        start=(c == 0), stop=(c == NCH - 1),
                    )
                psum_copy(ot[:, t, :], pt[:])
                if i == IMGS - 1:
                    # chunked final store: drain the tail as copies land
                    nc.scalar.dma_start(yr[i][:, t, :], ot[:, t, :])
            if i != IMGS - 1:
                # stores ride the ACT hwdge ring; loads own the SP ring
                nc.scalar.dma_start(yr[i], ot[:])
            xt[i] = tt[i] = None

        pass1(0)
        for i in range(1, IMGS):
            pass1(i)
            pass2(i - 1)
        pass2(IMGS - 1)

    nc.compile()
    return nc


def kernel(x, weight):
    from concourse.bass_utils import run_bass_kernel_spmd

    x = np.ascontiguousarray(x, dtype=np.float32)
    w2d = np.asarray(weight, dtype=np.float64)[:, 0]  # [C, 41, 41]

    # Extract the separable 1-D factor per channel; all channels share one
    # kernel in this model, so verify and collapse.
    g0 = w2d[0].sum(axis=1)
    assert np.allclose(w2d, w2d[0], rtol=0, atol=1e-12), "channels differ"
    assert np.allclose(np.outer(g0, g0), w2d[0], rtol=1e-6, atol=1e-12), \
        "kernel is not separable"

    A = _build_A(g0).astype(np.float32)
    min_n = 256 if MM_DTYPE == "f32r" else 1
    wins = _windows(A, min_n)
    wmax = max(hi - lo for lo, hi in wins)
    Ac = np.zeros((NCH, P, wmax), dtype=np.float32)
    for c, (lo, hi) in enumerate(wins):
        Ac[c, :, :hi - lo] = A[c * P:(c + 1) * P, lo:hi]

    key = (MM_DTYPE, tuple(wins))
    if key not in _cache:
        _cache[key] = _build_program(wins)
    nc = _cache[key]

    # shard batch over cores: core i gets batches [2i, 2i+1] -> 12 images
    xs = x.reshape(NCORES, IMGS, H, W)
    in_maps = [{"x": xs[i], "A": Ac} for i in range(NCORES)]
    res = run_bass_kernel_spmd(nc, in_maps, list(range(NCORES)))
    globals()["_last_run"] = res
    out = np.stack([res.results[i]["y"] for i in range(NCORES)])
    return out.reshape(B, C, H, W)


# revision 20
# speedup vs baseline: 1.0954x; 1.0954x over previous
"""Depthwise 41x41 Gaussian blur (ReflectionPad2d(20)) on 8 Trainium2 cores.

Strategy:
  * The 41x41 Gaussian kernel is rank-1 (separable): w2d = outer(g, g) with
    sum(g) == 1.  We extract g = w2d.sum(axis=1) on the host and run two 1-D
    41-tap convolution passes (along H, then along W).
  * Each 1-D pass is expressed as a banded matmul: out = A^T-contract over
    input rows, where A[r, h] = sum_k g[k] [reflect(h+k-20) == r] folds the
    reflection padding into the band matrix.  A is built on the host in
    float64 from the runtime weight.
  * On-device, pass 1 uses the image as the matmul stationary operand
    (lhsT = x[r_chunk, w_tile], rhs = A[r_chunk, h_window]) producing the
    H-blurred image TRANSPOSED ([w, h] layout); pass 2 repeats the same
    structure on the transposed intermediate, which lands the final image
    back in natural [h, w] layout.  No explicit transposes anywhere.
  * Band structure: for each 128-row input chunk only a ~208-wide window of
    output columns is touched (padded to 256 — the fp32r matmul runs at
    1 cyc/row only for N>=256, hardware-verified), so each PSUM accumulation
    group is 4 matmuls with clipped, overlapping N-windows (start=True
    clears the whole bank, so overwrite-then-accumulate per element is
    exact).  A is shipped band-compacted ([4, 128, 256]) to save DMA.
  * Sharding: data-parallel over batch*channel; 96 images / 8 cores = 12
    images per core.  The band matrix A is replicated.
"""

import numpy as np

B, C, H, W = 16, 6, 512, 512
PAD, KS = 20, 41
NCORES = 8
IMGS = (B * C) // NCORES  # 12 images per core
P = 128
NCH = H // P  # 4 chunks of 128 rows

# Matmul input dtype: "f32" (exact, 4 cyc/row) or "f32r" (TF32-like ~1.5e-4
# relative error, 1 cyc/row at N>=256).
MM_DTYPE = "f32r"

_cache = {}


def _reflect(r):
    # PyTorch ReflectionPad2d index map for pad=20 on a length-512 axis.
    if r < 0:
        return -r
    if r > H - 1:
        return 2 * (H - 1) - r
    return r


def _build_A(g):
    """A[r, h] = sum of g[k] over taps k whose padded input row reflects to r."""
    A = np.zeros((H, H), dtype=np.float64)
    for h in range(H):
        for k in range(KS):
            A[_reflect(h + k - PAD), h] += g[k]
    return A


def _windows(A, min_n):
    """Per-128-row-chunk nonzero output-column windows, padded to >= min_n."""
    wins = []
    for c in range(NCH):
        nz = np.nonzero(A[c * P:(c + 1) * P, :].any(axis=0))[0]
        lo, hi = int(nz[0]), int(nz[-1]) + 1
        if hi - lo < min_n:
            lo = max(0, hi - min_n)
            hi = min(H, lo + min_n)
            lo = max(0, hi - min_n)
        wins.append((lo, hi))
    # the union of windows must cover every output column exactly
    cover = np.zeros(H, dtype=bool)
    prev_hi = 0
    for lo, hi in wins:
        assert lo <= prev_hi, f"window gap: {wins}"
        cover[lo:hi] = True
        prev_hi = max(prev_hi, hi)
    assert cover.all(), f"windows don't cover output: {wins}"
    return wins


def _build_program(wins):
    from contextlib import ExitStack
    import concourse.tile as tile
    from concourse import bacc, mybir

    dt_in = mybir.dt.float32r if MM_DTYPE == "f32r" else mybir.dt.float32
    f32 = mybir.dt.float32

    nc = bacc.Bacc("TRN2", target_bir_lowering=False, debug=False,
                   num_devices=NCORES)
    wmax = max(hi - lo for lo, hi in wins)
    x_d = nc.dram_tensor("x", [IMGS, H, W], dt_in, kind="ExternalInput").ap()
    a_d = nc.dram_tensor("A", [NCH, P, wmax], dt_in, kind="ExternalInput").ap()
    y_d = nc.dram_tensor("y", [IMGS, H, W], f32, kind="ExternalOutput").ap()

    xr = x_d.rearrange("i (c p) w -> i p c w", p=P)
    yr = y_d.rearrange("i (c p) w -> i p c w", p=P)
    ar = a_d.rearrange("c p j -> p c j")

    with tile.TileContext(nc) as tc, ExitStack() as ctx:
        apool = ctx.enter_context(tc.tile_pool(name="a", bufs=1))
        xpool = ctx.enter_context(tc.tile_pool(name="x", bufs=6))
        tpool = ctx.enter_context(tc.tile_pool(name="t", bufs=3))
        opool = ctx.enter_context(tc.tile_pool(name="o", bufs=6))
        ps1 = ctx.enter_context(tc.tile_pool(name="ps1", bufs=4, space="PSUM"))
        ps2 = ctx.enter_context(tc.tile_pool(name="ps2", bufs=4, space="PSUM"))

        At = apool.tile([P, NCH, wmax], dt_in, tag="A")

        xt = [None] * IMGS
        tt = [None] * IMGS

        def psum_copy(dst, src):
            nc.vector.tensor_copy(dst, src)

        def pass1(i):
            xt[i] = xpool.tile([P, NCH, W], dt_in, tag="x", name=f"x{i}")
            if i == 0:
                # chunk-interleaved first loads so the PE can start after
                # ~0.5 MB of DMA instead of 2 MB
                for c in range(NCH):
                    nc.sync.dma_start(At[:, c, :], ar[:, c, :])
                    nc.sync.dma_start(xt[i][:, c, :], xr[i][:, c, :])
            elif i == IMGS - 1:
                # chunked last load shortens the serial tail
                for c in range(NCH):
                    nc.sync.dma_start(xt[i][:, c, :], xr[i][:, c, :])
            else:
                nc.sync.dma_start(xt[i][:], xr[i])
            tt[i] = tpool.tile([P, NCH, H], dt_in, tag="t", name=f"t{i}")
            # c-outer emission: all 4 accumulation groups open at once, so
            # only the final c's matmuls wait on the last-arriving chunk
            pts = [ps1.tile([P, H], f32, tag="p1", name=f"p1_{i}_{m}")
                   for m in range(NCH)]
            for c in range(NCH):
                lo, hi = wins[c]
                for m in range(NCH):
                    nc.tensor.matmul(
                        pts[m][:, lo:hi],
                        xt[i][:, c, m * P:(m + 1) * P],
                        At[:, c, 0:hi - lo],
                        start=(c == 0), stop=(c == NCH - 1),
                    )
            for m in range(NCH):
                psum_copy(tt[i][:, m, :], pts[m][:])

        def pass2(i):
            ot = opool.tile([P, NCH, W], f32, tag="o")
            for t in range(NCH):
                pt = ps2.tile([P, W], f32, tag="p2")
                for c in range(NCH):
                    lo, hi = wins[c]
                    nc.tensor.matmul(
                        pt[:, lo:hi],
                        tt[i][:, c, t * P:(t + 1) * P],
                        At[:, c, 0:hi - lo],
                        start=(c == 0), stop=(c == NCH - 1),
                    )
                psum_copy(ot[:, t, :], pt[:])
                if i == IMGS - 1:
                    # chunked final store: drain the tail as copies land
                    nc.scalar.dma_start(yr[i][:, t, :], ot[:, t, :])
            if i != IMGS - 1:
                # stores ride the ACT hwdge ring; loads own the SP ring
                nc.scalar.dma_start(yr[i], ot[:])
            xt[i] = tt[i] = None

        pass1(0)
        for i in range(1, IMGS):
            pass1(i)
            pass2(i - 1)
        pass2(IMGS - 1)

    nc.compile()
    return nc


def kernel(x, weight):
    from concourse.bass_utils import run_bass_kernel_spmd

    x = np.ascontiguousarray(x, dtype=np.float32)
    w2d = np.asarray(weight, dtype=np.float64)[:, 0]  # [C, 41, 41]

    # Extract the separable 1-D factor per channel; all channels share one
    # kernel in this model, so verify and collapse.
    g0 = w2d[0].sum(axis=1)
    assert np.allclose(w2d, w2d[0], rtol=0, atol=1e-12), "channels differ"
    assert np.allclose(np.outer(g0, g0), w2d[0], rtol=1e-6, atol=1e-12), \
        "kernel is not separable"

    A = _build_A(g0).astype(np.float32)
    min_n = 256 if MM_DTYPE == "f32r" else 1
    wins = _windows(A, min_n)
    wmax = max(hi - lo for lo, hi in wins)
    Ac = np.zeros((NCH, P, wmax), dtype=np.float32)
    for c, (lo, hi) in enumerate(wins):
        Ac[c, :, :hi - lo] = A[c * P:(c + 1) * P, lo:hi]

    key = (MM_DTYPE, tuple(wins))
    if key not in _cache:
        _cache[key] = _build_program(wins)
    nc = _cache[key]

    # shard batch over cores: core i gets batches [2i, 2i+1] -> 12 images
    xs = x.reshape(NCORES, IMGS, H, W)
    in_maps = [{"x": xs[i], "A": Ac} for i in range(NCORES)]
    res = run_bass_kernel_spmd(nc, in_maps, list(range(NCORES)))
    globals()["_last_run"] = res
    out = np.stack([res.results[i]["y"] for i in range(NCORES)])
    return out.reshape(B, C, H, W)
